# revision 30
# baseline (speedup 1.0000x reference)
"""GRU kernel for Trainium2, 8-way batch data-parallel, 2-group pipelined.

Problem: flow_x [64, 1024, 512, 1] -> GRU over T=512 steps, hidden=1024,
returns final hidden state [64, 1024, 1, 1].

Per core (8 sequences, no collectives):
  - The 8 independent sequences are split into two groups of 4 that run the
    recurrence phase-shifted by half a step: group A's elementwise chain
    (DVE + Act) overlaps group B's PE matmul burst and vice versa, hiding
    the ~1.8us per-step semaphore/engine-latency chain behind PE work.
  - T-layout everywhere: h lives as [128 partitions, k-slice, batch] bf16 and
    is the matmul moving operand directly; hidden index n = k*128 + p.
  - Recurrence burst per group per step: 24 m-tiles x 8 k-tiles of
    stationary bf16 weights, moving operand h.T [128, 4]. b2 is folded into
    the h-hat PSUM accumulation with one extra identity-stationary matmul, so
    the DVE chain is: (zr+xzr add on Pool) -> sigmoid -> {1-z, z*h, r*ph,
    +xh} -> tanh -> {wm*hh, h'=m1+m2 (bf16 write)}.
  - Input projections x@W1+b1 / x@W2+b2 for chunk i+1 are computed during
    chunk i's recurrence: one 8-matmul PSUM group per step is interleaved
    into the PE instruction stream; PSUM->SBUF copy (bias fused) on Act.
  - flow_x / weights are loaded with casting DMAs (gpsimd SWDGE f32->bf16),
    so no staging buffers or vector-engine conversion copies are needed.
"""

import sys

sys.path.insert(0, "/opt/trn_rl_repo")

import numpy as np

B, N, T = 64, 1024, 512
NCORES = 8
BLOC = B // NCORES  # 8 sequences per core
TC = 32  # timestep chunk
NCHUNK = T // TC  # 16
KT = N // 128  # 8 k-tiles
GB = 4  # batches per pipeline group
NPROJ = 24  # projection m-groups per chunk (16 for W1, 8 for W2)

_CACHE = {}
TRACE = False  # test-only: set True to capture an NTFF profile on the next call


def _build_nc(n_chunks=NCHUNK, tc_steps=TC):
    from contextlib import ExitStack

    import concourse.bacc as bacc
    import concourse.bass as bass
    import concourse.mybir as mybir
    import concourse.tile as tile

    f32 = mybir.dt.float32
    bf16 = mybir.dt.bfloat16
    AF = mybir.ActivationFunctionType
    MUL = mybir.AluOpType.mult
    ADD = mybir.AluOpType.add

    nc = bacc.Bacc("TRN2", target_bir_lowering=False, debug=False)

    fx = nc.dram_tensor("flow_x", [BLOC, N, T], f32, kind="ExternalInput")
    w1 = nc.dram_tensor("W1", [N, 2 * N], f32, kind="ExternalInput")
    b1 = nc.dram_tensor("b1", [2 * N], f32, kind="ExternalInput")
    w2 = nc.dram_tensor("W2", [N, N], f32, kind="ExternalInput")
    b2 = nc.dram_tensor("b2", [N], f32, kind="ExternalInput")
    ident = nc.dram_tensor("ident", [128, 128], f32, kind="ExternalInput")
    out = nc.dram_tensor("h_out", [BLOC, N], f32, kind="ExternalOutput")

    with tile.TileContext(nc) as tc:
        with ExitStack() as stk:
            const = stk.enter_context(tc.tile_pool(name="const", bufs=1))

            sb = stk.enter_context(tc.tile_pool(name="sb", bufs=2))
            psum = stk.enter_context(
                tc.tile_pool(name="psum", bufs=1, space=bass.MemorySpace.PSUM)
            )
            ppsum = stk.enter_context(
                tc.tile_pool(name="ppsum", bufs=2, space=bass.MemorySpace.PSUM)
            )

            # ---- weights / consts: f32 staged on parallel HWDGE queues,
            # bf16 copies derived on-chip ----
            w1b = const.tile([128, KT, 2 * N], bf16, tag="w1b")
            w2b = const.tile([128, KT, N], bf16, tag="w2b")
            with tc.tile_pool(name="wstage", bufs=1) as wstage:
                st1 = wstage.tile([128, KT, 2 * N], f32, tag="st1")
                nc.sync.dma_start(st1[:], w1[:].rearrange("(k p) c -> p k c", p=128))
                st2 = wstage.tile([128, KT, N], f32, tag="st2")
                nc.scalar.dma_start(
                    st2[:], w2[:].rearrange("(k p) c -> p k c", p=128)
                )
                nc.vector.tensor_copy(w1b[:], st1[:])
                nc.vector.tensor_copy(w2b[:], st2[:])
            identb = const.tile([128, 128], bf16, tag="identb")
            nc.gpsimd.dma_start(identb[:], ident[:])
            b1s = const.tile([128, 16], f32, tag="b1s")
            nc.sync.dma_start(b1s[:], b1[:].rearrange("(s p) -> p s", p=128))
            b2s = const.tile([128, 8], f32, tag="b2s")
            nc.sync.dma_start(b2s[:], b2[:].rearrange("(s p) -> p s", p=128))
            # b2 broadcast across batch as bf16 fold operand: b2tb[p, m, b]
            b2sb = const.tile([128, 8], bf16, tag="b2sb")
            nc.vector.tensor_copy(b2sb[:], b2s[:])
            b2tb = const.tile([128, 8, BLOC], bf16, tag="b2tb")
            for b in range(BLOC):
                nc.vector.tensor_copy(b2tb[:, :, b], b2sb[:])

            # ---- persistent state ----
            ht = {}
            for grp, gsl in (("A", slice(0, GB)), ("B", slice(GB, BLOC))):
                ht[grp] = const.tile(
                    [128, KT, GB], bf16, tag=f"ht{grp}", name=f"ht{grp}"
                )
                nc.vector.memset(ht[grp][:], 0.0)
            # projections, double-buffered by chunk parity: [p, {z,r,h}, m, b, t]
            x3 = [
                const.tile(
                    [128, 3, 8, BLOC, tc_steps], bf16, tag=f"x3_{s}", name=f"x3_{s}"
                )
                for s in range(2)
            ]

            GSL = {"A": slice(0, GB), "B": slice(GB, BLOC)}

            def wtile(gm, k):
                if gm < 16:
                    return w1b[:, k, gm * 128 : (gm + 1) * 128]
                return w2b[:, k, (gm - 16) * 128 : (gm - 15) * 128]


            xinP = [
                const.tile(
                    [128, KT, BLOC, tc_steps], f32, tag=f"xin{s}", name=f"xin{s}"
                )
                for s in range(2)
            ]
            xinB = [
                const.tile(
                    [128, KT, BLOC, tc_steps], bf16, tag=f"xinb{s}", name=f"xinb{s}"
                )
                for s in range(2)
            ]

            def conv_xin(parity, k):
                """One k-slice f32 -> bf16 conversion on DVE."""
                nc.vector.tensor_copy(xinB[parity][:, k], xinP[parity][:, k])

            def dma_xin(toff, parity, oob, eng=None):
                """One chunk of flow_x -> f32 [p, k, b, t] via HWDGE DMAs,
                double-buffered by chunk parity (prefetched a chunk ahead).
                Consumed as an f32r moving operand (full rate at >=256 cols)."""
                if isinstance(toff, int) and toff >= T:
                    return
                x = xinP[parity]
                if eng is None:
                    eng = nc.sync
                for k in range(KT):
                    eng.dma_start(
                        x[:, k],
                        fx[:, k * 128 : (k + 1) * 128, bass.ds(toff, tc_steps)]
                        .rearrange("b p t -> p b t"),
                        bounds_check="skip_entire_dma"
                        if (oob and not isinstance(toff, int))
                        else None,
                    )

            def proj_mms(xtile, gm):
                """x[chunk] @ W[:, gm-slice] -> PSUM (one m-group)."""
                pp = ppsum.tile([128, BLOC, tc_steps], f32, tag="pp")
                for k in range(KT):
                    nc.tensor.matmul(
                        pp[:],
                        wtile(gm, k),
                        xtile[:, k],
                        start=(k == 0),
                        stop=(k == KT - 1),
                    )
                return pp

            def proj_copy(pp, gm, x3dst):
                bias = b1s[:, gm : gm + 1] if gm < 16 else b2s[:, gm - 16 : gm - 15]
                # Act/DVE alternating (never Pool: GPSIMD cannot read PSUM)
                if gm % 2 == 0:
                    nc.scalar.activation(
                        x3dst[:, gm // 8, gm % 8], pp[:], AF.Identity, bias=bias
                    )
                else:
                    nc.vector.tensor_scalar(
                        out=x3dst[:, gm // 8, gm % 8], in0=pp[:],
                        scalar1=bias, scalar2=None, op0=ADD,
                    )

            def proj_group(xtile, gm, x3dst, t0):
                proj_copy(proj_mms(xtile, gm), gm, x3dst)

            def burst(grp, slot, t):
                """All 24 m-tile matmul groups for one group's step. The xz/xr
                projections and b2 are folded into the PSUM accumulations via
                identity-stationary matmuls, so the activations read PSUM
                directly (no separate adds)."""
                h = ht[grp]
                gsl = GSL[grp]
                xs = x3[slot]
                zp = psum.tile([128, 8, GB], f32, tag=f"zp{grp}")
                rp = psum.tile([128, 8, GB], f32, tag=f"rp{grp}")
                ph = psum.tile([128, 8, GB], f32, tag=f"ph{grp}")

                def msec(dst, gm, fold_mv):
                    for k in range(KT):
                        nc.tensor.matmul(
                            dst, wtile(gm, k), h[:, k],
                            start=(k == 0), stop=False,
                        )
                    nc.tensor.matmul(dst, identb[:], fold_mv, start=False, stop=True)

                # r section first (gates s2, the head of the chain), then
                # h-hat (s2's other input), z last (needed later in the chain)
                for m in range(8):
                    msec(rp[:, m], 8 + m, xs[:, 1, m, gsl, t])
                for m in range(8):
                    msec(ph[:, m], 16 + m, b2tb[:, m, gsl])
                for m in range(8):
                    msec(zp[:, m], m, xs[:, 0, m, gsl, t])
                return (zp, rp), ph

            P = 4000  # virtual ns per step, for manual schedule control
            OFS = 100000  # clears the prologue's virtual-clock head start
            NOW = [None]

            def W(ns):
                """Pin following instructions to a virtual schedule floor."""
                if ns is not None:
                    tc.tile_set_cur_wait(ns / 1e6)
                NOW[0] = ns

            def chain_front(grp, zr, ph, slot, t, t0, base):
                """sigR off PSUM -> s2 = r*ph -> s3 = s2 + xh."""
                gsl = GSL[grp]
                xs = x3[slot]
                eng = nc.vector if grp == "A" else nc.gpsimd
                zp, rp = zr
                W(t0 + base + 450)
                zrs = sb.tile([128, 2, 8, GB], f32, tag=f"zrs{grp}")
                nc.scalar.activation(zrs[:, 1], rp[:], AF.Sigmoid)
                s2 = sb.tile([128, 8, GB], f32, tag=f"s2{grp}")
                # PSUM reads must stay off Pool: s2 always runs on DVE
                nc.vector.tensor_mul(s2[:], zrs[:, 1], ph[:])
                s3 = sb.tile([128, 8, GB], f32, tag=f"s3{grp}")
                eng.tensor_add(s3[:], s2[:], xs[:, 2, :, gsl, t])
                return zrs, s3

            def chain_mid(grp, zr, zrs, t0, base):
                """sigZ off PSUM -> m1 = z*h (h read before this step's write)."""
                zp, rp = zr
                eng = nc.vector if grp == "A" else nc.gpsimd
                W(t0 + base + 800)
                nc.scalar.activation(zrs[:, 0], zp[:], AF.Sigmoid)
                m1 = sb.tile([128, 8, GB], f32, tag=f"m1{grp}")
                eng.tensor_mul(m1[:], zrs[:, 0], ht[grp][:])
                return m1

            def chain_back(grp, zrs, s3, m1, t0, base):
                """tanh -> zm = (z-1)*hh -> h' = m1 - zm (bf16, on DVE)."""
                W(t0 + base + 1250)
                hh = sb.tile([128, 8, GB], f32, tag=f"hh{grp}")
                nc.scalar.activation(hh[:], s3[:], AF.Tanh)
                zm = sb.tile([128, 8, GB], f32, tag=f"zm{grp}")
                nc.vector.scalar_tensor_tensor(
                    zm[:], zrs[:, 0], 1.0, hh[:],
                    op0=mybir.AluOpType.subtract, op1=MUL,
                )
                h = ht[grp]
                nc.vector.tensor_sub(h[:, 0:4], m1[:, 0:4], zm[:, 0:4])
                nc.vector.tensor_sub(h[:, 4:8], m1[:, 4:8], zm[:, 4:8])

            def chunk_steps(slot, g0, dma_spec, do_proj, ofs=None):
                """One chunk of recurrence with a manually pinned schedule.
                Projections for chunk following the one being DMA-prefetched
                are interleaved one m-group per step; the flow_x DMA for two
                chunks ahead is issued at chunk start (full-chunk prefetch)."""
                if ofs is None:
                    ofs = OFS
                if dma_spec is not None:
                    W(ofs + g0 * P)
                    dma_xin(*dma_spec)
                xsrc = xinB[1 - slot]
                for t in range(tc_steps):
                    t0 = ofs + (g0 + t) * P
                    W(t0)
                    za, pa = burst("A", slot, t)
                    pp = None
                    if do_proj and t < NPROJ:
                        W(t0 + 950)
                        pp = proj_mms(xsrc, t)
                    if do_proj and t >= tc_steps - KT:
                        # convert next-next chunk's k-slice (DVE has slack
                        # here: these steps carry no projection groups)
                        conv_xin(slot, t - (tc_steps - KT))
                    W(t0 + 1800)
                    zb, pb = burst("B", slot, t)
                    zrs_a, s3a = chain_front("A", za, pa, slot, t, t0, 0)
                    tc.no_sync_barrier()
                    # tanhA first in the shared-Act contention window, then
                    # B's front; sigZ ops float into natural Act gaps
                    m1a = chain_mid("A", za, zrs_a, t0, 0)
                    chain_back("A", zrs_a, s3a, m1a, t0, 0)
                    zrs_b, s3b = chain_front("B", zb, pb, slot, t, t0, 1000)
                    m1b = chain_mid("B", zb, zrs_b, t0, 1000)
                    chain_back("B", zrs_b, s3b, m1b, t0, 1000)
                    if pp is not None:
                        W(t0 + 2900)
                        proj_copy(pp, t, x3[1 - slot])

            # ---- prologue: chunk 0 + 1 data, chunk 0 projections ----
            dma_xin(0, 0, oob=False, eng=nc.scalar)
            for k in range(KT):
                conv_xin(0, k)
            for g in range(NPROJ):
                proj_group(xinB[0], g, x3[0], None)
            if n_chunks > 1:
                dma_xin(tc_steps, 1, oob=False)
                for k in range(KT):
                    conv_xin(1, k)

            # ---- main loop: two chunks per iteration (static slot parity) ----
            def body_pair(j, ofs=None):
                c2 = j * (2 * tc_steps) + 2 * tc_steps
                chunk_steps(0, 0, (c2, 0, True), do_proj=True, ofs=ofs)
                chunk_steps(
                    1, tc_steps, (c2 + tc_steps, 1, True), do_proj=True, ofs=ofs
                )

            npair = n_chunks // 2
            if npair > 2:
                # last pair peeled: its prefetch DMAs are statically OOB
                with tc.For_i(0, npair - 1, 1) as j:
                    body_pair(j)
                body_pair(npair - 1, ofs=OFS + 80 * P)
            elif npair >= 1:
                for j in range(npair):
                    body_pair(j)
            else:  # n_chunks == 1 (mini): no pipelining, just run chunk 0
                chunk_steps(0, 0, None, do_proj=False)
            W(OFS + 160 * tc_steps * P)

            # ---- epilogue: h -> DRAM ----
            hfin = const.tile([128, 8, BLOC], f32, tag="hfin")
            nc.vector.tensor_copy(hfin[:, :, 0:GB], ht["A"][:])
            nc.vector.tensor_copy(hfin[:, :, GB:BLOC], ht["B"][:])
            for m in range(8):
                nc.sync.dma_start(
                    out[:, m * 128 : (m + 1) * 128].rearrange("b p -> p b"),
                    hfin[:, m, :],
                )

    nc.compile()
    return nc


def kernel(flow_x, W1, b1, W2, b2):
    from concourse.bass_utils import run_bass_kernel_spmd

    if "nc" not in _CACHE:
        _CACHE["nc"] = _build_nc()
    nc = _CACHE["nc"]

    fx = np.ascontiguousarray(flow_x.reshape(B, N, T).astype(np.float32))
    eye = np.ascontiguousarray(np.eye(128, dtype=np.float32))
    in_maps = []
    for c in range(NCORES):
        in_maps.append(
            {
                "flow_x": fx[c * BLOC : (c + 1) * BLOC],
                "W1": np.ascontiguousarray(W1.astype(np.float32)),
                "b1": np.ascontiguousarray(b1.astype(np.float32)),
                "W2": np.ascontiguousarray(W2.astype(np.float32)),
                "b2": np.ascontiguousarray(b2.astype(np.float32)),
                "ident": eye,
            }
        )
    kw = {}
    if TRACE:
        kw = {"trace": True}
    res = run_bass_kernel_spmd(nc, in_maps, list(range(NCORES)), **kw)
    _CACHE["last_res"] = res
    outs = [res.results[c]["h_out"] for c in range(NCORES)]
    h = np.concatenate(outs, axis=0)
    return h.reshape(B, N, 1, 1).astype(np.float32)
